# revision 17
# baseline (speedup 1.0000x reference)
"""Bahdanau-style attention scoring kernel for 8 TRN2 NeuronCores.

Reference computation (B=128, H=256, N=2048):
    hidden = concat([static, dynamic, broadcast(dec)], axis=1)   # [B, 3H, N]
    scores = tanh(einsum('hk,bkn->bhn', W[0], hidden))           # [B, H, N]
    logits = einsum('h,bhn->bn', v[0,0], scores)                 # [B, N]
    attns  = softmax(logits, axis=-1)[:, None, :]                # [B, 1, N]

The broadcast decoder term collapses to a per-batch bias vector
c[b] = W_dec @ dec[b] (precomputed on host, 0.003% of FLOPs), so per batch:
    scores_b = tanh(W_s @ static[b] + W_d @ dynamic[b] + c[b])

Sharding: data-parallel over batch, 16 batches per core; tiny W/v params
replicated (pre-cast to bf16 on host). No collectives needed.

Matmuls run in bf16 (1 cycle/row on the PE, f32 PSUM accumulate; verified
rel err ~1.9e-3 vs the 2e-2 gate). The 64MB/core of f32 activations never
touches a compute engine on the way in: the DMA reads only the high 2 bytes
of each f32 (little-endian offset +2, stride 4) which IS bf16 truncation —
the cast is free.

Weight-stationary loop order (m -> kt -> nt) into a [128, 4x512] PSUM
supertile cuts LDWEIGHTS count 4x vs kt-inner order.

The v-reduction uses a masked stationary matrix vm[p, b, m, j] =
v[m*128+p] * (j==b) so batch b's logits land on PSUM partition b,
accumulating all 16 batches into one [16, 512] PSUM tile per n-tile.
Softmax then runs batch-parallel on partitions 0..15.
"""

import sys

if "/opt/trn_rl_repo" not in sys.path:
    sys.path.insert(0, "/opt/trn_rl_repo")

import numpy as np

B, H, N = 128, 256, 2048
NCORES = 8
BPC = B // NCORES  # batches per core
P = 128            # SBUF partitions
KT = 4             # k-tiles over 2H=512 contraction
MT = 2             # m-tiles over H=256 output rows
NS = 512           # n-tile (one PSUM bank of f32)
NT = N // NS       # 4 n-tiles

_CACHE = {}


def _build():
    import concourse.bacc as bacc
    from concourse import mybir
    from concourse.tile import TileContext

    f32 = mybir.dt.float32
    bf16 = mybir.dt.bfloat16
    Tanh = mybir.ActivationFunctionType.Tanh
    Exp = mybir.ActivationFunctionType.Exp

    nc = bacc.Bacc()
    xs = nc.declare_dram_parameter("xs", [BPC, H, N], f32, isOutput=False)
    xd = nc.declare_dram_parameter("xd", [BPC, H, N], f32, isOutput=False)
    # wt[k, h] = W[h, k] for k in [0, 512): rows 0:256 static, 256:512 dynamic
    wt = nc.declare_dram_parameter("wt", [2 * H, H], bf16, isOutput=False)
    # cb[h, b] = sum_k W[h, 512+k] * dec[b, k]  (host-precomputed bias)
    cb = nc.declare_dram_parameter("cb", [H, BPC], f32, isOutput=False)
    # vm[p, b, m, j] = v[m*128 + p] * (j == b)
    vm = nc.declare_dram_parameter("vm", [P, BPC, MT, BPC], bf16, isOutput=False)
    out = nc.declare_dram_parameter("out", [BPC, N], f32, isOutput=True)

    with (
        TileContext(nc) as tc,
        tc.tile_pool(name="const", bufs=1) as cpool,
        tc.tile_pool(name="xh", bufs=5) as hpool,
        tc.tile_pool(name="sc", bufs=2) as spool,
        tc.tile_pool(name="ps", bufs=2, space="PSUM") as ppool,
        tc.tile_pool(name="pl", bufs=1, space="PSUM") as plpool,
    ):
        # x loads: one DMA per (batch, source), issued two batches ahead;
        # batch 0/1 loads go out before the parameter DMAs so the PE's
        # first matmul isn't stuck behind them on the sync sequencer.
        xf_tiles = {}

        def issue_x_dmas(bb):
            tiles = []
            for kt in range(2):
                xf = hpool.tile([P, N], f32, name=f"xsf{kt}", tag=f"xsf{kt}")
                nc.sync.dma_start(out=xf[:], in_=xs[bb, kt * P:(kt + 1) * P, :])
                tiles.append(xf)
            for kt in range(2):
                xf = hpool.tile([P, N], f32, name=f"xdf{kt}", tag=f"xdf{kt}")
                nc.sync.dma_start(out=xf[:], in_=xd[bb, kt * P:(kt + 1) * P, :])
                tiles.append(xf)
            xf_tiles[bb] = tiles

        # --- replicated parameters, one simple DMA per separate tile ---
        wt_sb = []
        for kt in range(KT):
            w = cpool.tile([P, H], bf16, name=f"wt{kt}", tag=f"wt{kt}")
            nc.gpsimd.dma_start(out=w[:], in_=wt[kt * P:(kt + 1) * P, :])
            wt_sb.append(w)
        vm_sb = cpool.tile([P, BPC, MT, BPC], bf16)
        nc.gpsimd.dma_start(out=vm_sb[:], in_=vm[:])
        # bias laid out [128, m, b]
        c_sb = cpool.tile([P, MT, BPC], f32)
        nc.gpsimd.dma_start(out=c_sb[:], in_=cb[:].rearrange("(m p) b -> p m b", p=P))

        issue_x_dmas(0)
        issue_x_dmas(1)

        # logits accumulators: one [BPC, 512] PSUM tile per n-tile, written by
        # all 16 batches' masked v-matmuls (batch b lands on partition b)
        lp_tiles = [
            plpool.tile([BPC, NS], f32, tag=f"lp{nt}", name=f"lp{nt}")
            for nt in range(NT)
        ]

        # --- main loop: 16 batches; v-matmuls are software-pipelined one
        # batch behind the main matmuls so the PE never waits on the
        # scalar engine's tanh. Emission orders are chosen so the first
        # instruction of each group carries the group's maximal semaphore
        # wait, letting Tile elide the rest (each satisfied-but-emitted
        # wait costs ~350ns of PE sequencer time):
        #   - tanh per m-group runs nt = 3..0, so the last tanh tick is nt0
        #   - v-matmuls start at (nt0, m1), mains at (kt0, nt0)
        sc_hist = {}

        def emit_vmms(vb):
            sc_prev = sc_hist.pop(vb)
            for nt in range(NT):
                for m in (1, 0):
                    nc.tensor.matmul(
                        lp_tiles[nt][:],
                        lhsT=vm_sb[:, vb, m, :],
                        rhs=sc_prev[:, m, nt * NS:(nt + 1) * NS],
                        start=(vb == 0 and m == 1),
                        stop=(vb == BPC - 1 and m == 0),
                    )

        for b in range(BPC):
            if b + 2 < BPC:
                issue_x_dmas(b + 2)
            xh = [t[:].bitcast(bf16)[:, 1::2] for t in xf_tiles.pop(b)]

            # weight-stationary matmuls into 2-bank PSUM half-supertiles
            # (bufs=2 -> the slot-reuse WAR lands half a batch back and is
            # always satisfied, so the PE never stalls on tanh)
            sc_t = spool.tile([P, MT, N], bf16, tag="sc")
            for m in range(MT):
                for nh in range(2):
                    pst = ppool.tile([P, 2, NS], f32, tag="pst")
                    for kt in range(KT):
                        for nt2 in range(2):
                            nt = nh * 2 + nt2
                            nc.tensor.matmul(
                                pst[:, nt2, :],
                                lhsT=wt_sb[kt][:, m * P:(m + 1) * P],
                                rhs=xh[kt][:, nt * NS:(nt + 1) * NS],
                                start=(kt == 0),
                                stop=(kt == KT - 1),
                            )
                    for nt2 in reversed(range(2)):
                        nt = nh * 2 + nt2
                        nc.scalar.activation(
                            sc_t[:, m, nt * NS:(nt + 1) * NS], pst[:, nt2, :], Tanh,
                            bias=c_sb[:, m, b:b + 1],
                        )
            sc_hist[b] = sc_t
            if b > 0:
                emit_vmms(b - 1)
        emit_vmms(BPC - 1)

        # --- softmax over N per batch row (no max-subtraction: |logits| <~ 10) ---
        exp_sb = cpool.tile([BPC, N], f32)
        psums = cpool.tile([BPC, NT], f32)
        for nt in range(NT):
            nc.scalar.activation(
                exp_sb[:, nt * NS:(nt + 1) * NS], lp_tiles[nt][:], Exp,
                accum_out=psums[:, nt:nt + 1],
            )
        ssum = cpool.tile([BPC, 1], f32)
        nc.vector.reduce_sum(ssum[:], psums[:], axis=mybir.AxisListType.X)
        rec = cpool.tile([BPC, 1], f32)
        nc.vector.reciprocal(rec[:], ssum[:])
        nc.vector.tensor_scalar_mul(exp_sb[:], exp_sb[:], rec[:])
        nc.sync.dma_start(out=out[:], in_=exp_sb[:])

    nc.compile()
    return nc


def _make_in_maps(static_hidden, dynamic_hidden, decoder_hidden, v, W):
    import ml_dtypes

    bf16 = ml_dtypes.bfloat16
    W0 = np.asarray(W, dtype=np.float32)[0]          # [256, 768]
    wt_np = np.ascontiguousarray(W0[:, :2 * H].T.astype(bf16))   # [512, 256]
    vhalf = np.asarray(v, dtype=np.float32)[0, 0].reshape(MT, P)  # [2, 128]
    # vm[p, b, m, j] = v[m*128+p] * (j == b)
    vm_np = np.ascontiguousarray(
        np.einsum("mp,bj->pbmj", vhalf, np.eye(BPC, dtype=np.float32))
        .astype(bf16)
    )

    sh = np.asarray(static_hidden, dtype=np.float32)
    dh = np.asarray(dynamic_hidden, dtype=np.float32)
    dec = np.asarray(decoder_hidden, dtype=np.float32)
    # cb[h, b] = sum_k W_dec[h, k] dec[b, k], fp32 on host (tiny)
    cb_full = W0[:, 2 * H:] @ dec.T                  # [256, B]

    in_maps = []
    for i in range(NCORES):
        sl = slice(i * BPC, (i + 1) * BPC)
        in_maps.append({
            "xs": np.ascontiguousarray(sh[sl]),
            "xd": np.ascontiguousarray(dh[sl]),
            "wt": wt_np,
            "cb": np.ascontiguousarray(cb_full[:, sl]),
            "vm": vm_np,
        })
    return in_maps


def kernel(static_hidden, dynamic_hidden, decoder_hidden, v, W):
    from concourse.bass_utils import run_bass_kernel_spmd

    if "nc" not in _CACHE:
        _CACHE["nc"] = _build()
    nc = _CACHE["nc"]

    in_maps = _make_in_maps(static_hidden, dynamic_hidden, decoder_hidden, v, W)
    res = run_bass_kernel_spmd(nc, in_maps, core_ids=list(range(NCORES)))
    out = np.concatenate([r["out"] for r in res.results], axis=0)
    return out.reshape(B, 1, N).astype(np.float32)


# revision 18
# speedup vs baseline: 1.0247x; 1.0247x over previous
"""Bahdanau-style attention scoring kernel for 8 TRN2 NeuronCores.

Reference computation (B=128, H=256, N=2048):
    hidden = concat([static, dynamic, broadcast(dec)], axis=1)   # [B, 3H, N]
    scores = tanh(einsum('hk,bkn->bhn', W[0], hidden))           # [B, H, N]
    logits = einsum('h,bhn->bn', v[0,0], scores)                 # [B, N]
    attns  = softmax(logits, axis=-1)[:, None, :]                # [B, 1, N]

The broadcast decoder term collapses to a per-batch bias vector
c[b] = W_dec @ dec[b] (precomputed on host, 0.003% of FLOPs), so per batch:
    scores_b = tanh(W_s @ static[b] + W_d @ dynamic[b] + c[b])

Sharding: data-parallel over batch, 16 batches per core; tiny W/v params
replicated (pre-cast to bf16 on host). No collectives needed.

Matmuls run in bf16 (1 cycle/row on the PE, f32 PSUM accumulate; verified
rel err ~1.9e-3 vs the 2e-2 gate). The 64MB/core of f32 activations never
touches a compute engine on the way in: the DMA reads only the high 2 bytes
of each f32 (little-endian offset +2, stride 4) which IS bf16 truncation —
the cast is free.

Weight-stationary loop order (m -> kt -> nt) into a [128, 4x512] PSUM
supertile cuts LDWEIGHTS count 4x vs kt-inner order.

The v-reduction uses a masked stationary matrix vm[p, b, m, j] =
v[m*128+p] * (j==b) so batch b's logits land on PSUM partition b,
accumulating all 16 batches into one [16, 512] PSUM tile per n-tile.
Softmax then runs batch-parallel on partitions 0..15.
"""

import sys

if "/opt/trn_rl_repo" not in sys.path:
    sys.path.insert(0, "/opt/trn_rl_repo")

import numpy as np

B, H, N = 128, 256, 2048
NCORES = 8
BPC = B // NCORES  # batches per core
P = 128            # SBUF partitions
KT = 4             # k-tiles over 2H=512 contraction
MT = 2             # m-tiles over H=256 output rows
NS = 512           # n-tile (one PSUM bank of f32)
NT = N // NS       # 4 n-tiles

_CACHE = {}


def _build():
    import concourse.bacc as bacc
    from concourse import mybir
    from concourse.tile import TileContext

    f32 = mybir.dt.float32
    bf16 = mybir.dt.bfloat16
    Tanh = mybir.ActivationFunctionType.Tanh
    Exp = mybir.ActivationFunctionType.Exp

    nc = bacc.Bacc()
    xs = nc.declare_dram_parameter("xs", [BPC, H, N], f32, isOutput=False)
    xd = nc.declare_dram_parameter("xd", [BPC, H, N], f32, isOutput=False)
    # wt[k, h] = W[h, k] for k in [0, 512): rows 0:256 static, 256:512 dynamic
    wt = nc.declare_dram_parameter("wt", [2 * H, H], bf16, isOutput=False)
    # cb[h, b] = sum_k W[h, 512+k] * dec[b, k]  (host-precomputed bias)
    cb = nc.declare_dram_parameter("cb", [H, BPC], f32, isOutput=False)
    # vm[p, b, m, j] = v[m*128 + p] * (j == b)
    vm = nc.declare_dram_parameter("vm", [P, BPC, MT, BPC], bf16, isOutput=False)
    out = nc.declare_dram_parameter("out", [BPC, N], f32, isOutput=True)

    with (
        TileContext(nc) as tc,
        tc.tile_pool(name="const", bufs=1) as cpool,
        tc.tile_pool(name="xh", bufs=5) as hpool,
        tc.tile_pool(name="sc", bufs=2) as spool,
        tc.tile_pool(name="ps", bufs=2, space="PSUM") as ppool,
        tc.tile_pool(name="pl", bufs=1, space="PSUM") as plpool,
    ):
        # x loads: one DMA per (batch, source), issued two batches ahead;
        # batch 0/1 loads go out before the parameter DMAs so the PE's
        # first matmul isn't stuck behind them on the sync sequencer.
        xf_tiles = {}

        def issue_x_dmas(bb):
            tiles = []
            for kt in range(2):
                xf = hpool.tile([P, N], f32, name=f"xsf{kt}", tag=f"xsf{kt}")
                nc.sync.dma_start(out=xf[:], in_=xs[bb, kt * P:(kt + 1) * P, :])
                tiles.append(xf)
            for kt in range(2):
                xf = hpool.tile([P, N], f32, name=f"xdf{kt}", tag=f"xdf{kt}")
                nc.sync.dma_start(out=xf[:], in_=xd[bb, kt * P:(kt + 1) * P, :])
                tiles.append(xf)
            xf_tiles[bb] = tiles

        # --- replicated parameters, one simple DMA per separate tile ---
        wt_sb = []
        for kt in range(KT):
            w = cpool.tile([P, H], bf16, name=f"wt{kt}", tag=f"wt{kt}")
            nc.gpsimd.dma_start(out=w[:], in_=wt[kt * P:(kt + 1) * P, :])
            wt_sb.append(w)
        vm_sb = cpool.tile([P, BPC, MT, BPC], bf16)
        nc.gpsimd.dma_start(out=vm_sb[:], in_=vm[:])
        # bias laid out [128, m, b]
        c_sb = cpool.tile([P, MT, BPC], f32)
        nc.gpsimd.dma_start(out=c_sb[:], in_=cb[:].rearrange("(m p) b -> p m b", p=P))

        issue_x_dmas(0)
        issue_x_dmas(1)

        # logits accumulators: one [BPC, 512] PSUM tile per n-tile, written by
        # all 16 batches' masked v-matmuls (batch b lands on partition b)
        lp_tiles = [
            plpool.tile([BPC, NS], f32, tag=f"lp{nt}", name=f"lp{nt}")
            for nt in range(NT)
        ]

        # --- main loop: 16 batches; v-matmuls are software-pipelined one
        # batch behind the main matmuls so the PE never waits on the
        # scalar engine's tanh. Emission orders are chosen so the first
        # instruction of each group carries the group's maximal semaphore
        # wait, letting Tile elide the rest (each satisfied-but-emitted
        # wait costs ~350ns of PE sequencer time):
        #   - tanh per m-group runs nt = 3..0, so the last tanh tick is nt0
        #   - v-matmuls start at (nt0, m1), mains at (kt0, nt0)
        sc_hist = {}

        def emit_vmms(vb):
            sc_prev = sc_hist.pop(vb)
            for nt in range(NT):
                for m in (1, 0):
                    nc.tensor.matmul(
                        lp_tiles[nt][:],
                        lhsT=vm_sb[:, vb, m, :],
                        rhs=sc_prev[:, m, nt * NS:(nt + 1) * NS],
                        start=(vb == 0 and m == 1),
                        stop=(vb == BPC - 1 and m == 0),
                    )

        for b in range(BPC):
            if b + 2 < BPC:
                issue_x_dmas(b + 2)
            xh = [t[:].bitcast(bf16)[:, 1::2] for t in xf_tiles.pop(b)]

            # weight-stationary matmuls into 2-bank PSUM half-supertiles
            # (bufs=2 -> the slot-reuse WAR lands half a batch back and is
            # always satisfied, so the PE never stalls on tanh)
            sc_t = spool.tile([P, MT, N], bf16, tag="sc")
            for m in range(MT):
                for nh in range(2):
                    pst = ppool.tile([P, 2, NS], f32, tag="pst")
                    for kt in range(KT):
                        for nt2 in range(2):
                            nt = nh * 2 + nt2
                            nc.tensor.matmul(
                                pst[:, nt2, :],
                                lhsT=wt_sb[kt][:, m * P:(m + 1) * P],
                                rhs=xh[kt][:, nt * NS:(nt + 1) * NS],
                                start=(kt == 0),
                                stop=(kt == KT - 1),
                            )
                    for nt2 in reversed(range(2)):
                        nt = nh * 2 + nt2
                        nc.scalar.activation(
                            sc_t[:, m, nt * NS:(nt + 1) * NS], pst[:, nt2, :], Tanh,
                            bias=c_sb[:, m, b:b + 1],
                        )
            sc_hist[b] = sc_t
            if b > 0:
                emit_vmms(b - 1)
        emit_vmms(BPC - 1)

        # --- softmax over N per batch row (no max-subtraction: |logits| <~ 10) ---
        exp_sb = cpool.tile([BPC, N], f32)
        psums = cpool.tile([BPC, NT], f32)
        for nt in range(NT):
            nc.scalar.activation(
                exp_sb[:, nt * NS:(nt + 1) * NS], lp_tiles[nt][:], Exp,
                accum_out=psums[:, nt:nt + 1],
            )
        ssum = cpool.tile([BPC, 1], f32)
        nc.vector.reduce_sum(ssum[:], psums[:], axis=mybir.AxisListType.X)
        rec = cpool.tile([BPC, 1], f32)
        nc.vector.reciprocal(rec[:], ssum[:])
        for nt in range(NT):
            ns = slice(nt * NS, (nt + 1) * NS)
            nc.vector.tensor_scalar_mul(exp_sb[:, ns], exp_sb[:, ns], rec[:])
            nc.sync.dma_start(out=out[:, ns], in_=exp_sb[:, ns])

    nc.compile()
    return nc


def _make_in_maps(static_hidden, dynamic_hidden, decoder_hidden, v, W):
    import ml_dtypes

    bf16 = ml_dtypes.bfloat16
    W0 = np.asarray(W, dtype=np.float32)[0]          # [256, 768]
    wt_np = np.ascontiguousarray(W0[:, :2 * H].T.astype(bf16))   # [512, 256]
    vhalf = np.asarray(v, dtype=np.float32)[0, 0].reshape(MT, P)  # [2, 128]
    # vm[p, b, m, j] = v[m*128+p] * (j == b)
    vm_np = np.ascontiguousarray(
        np.einsum("mp,bj->pbmj", vhalf, np.eye(BPC, dtype=np.float32))
        .astype(bf16)
    )

    sh = np.asarray(static_hidden, dtype=np.float32)
    dh = np.asarray(dynamic_hidden, dtype=np.float32)
    dec = np.asarray(decoder_hidden, dtype=np.float32)
    # cb[h, b] = sum_k W_dec[h, k] dec[b, k], fp32 on host (tiny)
    cb_full = W0[:, 2 * H:] @ dec.T                  # [256, B]

    in_maps = []
    for i in range(NCORES):
        sl = slice(i * BPC, (i + 1) * BPC)
        in_maps.append({
            "xs": np.ascontiguousarray(sh[sl]),
            "xd": np.ascontiguousarray(dh[sl]),
            "wt": wt_np,
            "cb": np.ascontiguousarray(cb_full[:, sl]),
            "vm": vm_np,
        })
    return in_maps


def kernel(static_hidden, dynamic_hidden, decoder_hidden, v, W):
    from concourse.bass_utils import run_bass_kernel_spmd

    if "nc" not in _CACHE:
        _CACHE["nc"] = _build()
    nc = _CACHE["nc"]

    in_maps = _make_in_maps(static_hidden, dynamic_hidden, decoder_hidden, v, W)
    res = run_bass_kernel_spmd(nc, in_maps, core_ids=list(range(NCORES)))
    out = np.concatenate([r["out"] for r in res.results], axis=0)
    return out.reshape(B, 1, N).astype(np.float32)
